# revision 1
# baseline (speedup 1.0000x reference)
"""Multi-head attention (B=1, S=4096, D=1024, H=16) on 8 TRN2 NeuronCores.

Sharding: tensor-parallel over heads — 2 heads per core. W_q/W_k/W_v are
column-sharded (rows of the torch-style weight), W_o row-sharded; each core
produces a partial output [S, D] and the host sums the 8 partials.

Per-core dataflow (all in fp16 except f32 PSUM accumulation / output):
  1. QcT/KcT = [128(2h*64), 4096]: Qc^T = Wq_c @ q^T via chunked matmuls.
  2. Vc (natural [s, j] layout) + augmented ones column per head (gives the
     softmax denominator for free through the AV matmul).
  3. Attention in S^T layout: St[sk, sq] = Kh^T.T @ Qh^T (two heads packed via
     PE row-tiling), exp via ScalarE directly from PSUM (scale=1/8 and a
     constant -8 shift folded in; max-subtraction skipped — scores bounded),
     AV: U[65, sq] += [Vh | 1]^T @ E accumulated over all sk blocks.
  4. Row 64 of U is the denominator; reciprocal + K=1-matmul broadcast +
     DVE multiply normalizes into CT (= C^T), exactly the lhsT layout the
     output projection needs. Out-proj partials DMA straight from PSUM.

Softmax bias subtlety: a nonzero b_q adds a per-COLUMN (sk) offset
c_h[sk] = K_h[sk]·b_q_h to the scores (row-constant terms cancel in softmax).
This is handled exactly by scaling V rows and the ones-column by exp(c_h[sk])
(host passes the tiny exp(c) vectors; all-ones when b_q == 0).
b_v/b_o contribute a constant row vector w_o@b_v + b_o added on the host
(softmax rows sum to 1).
"""

import sys

sys.path.insert(0, "/opt/trn_rl_repo")

import numpy as np

import concourse.bass as bass
import concourse.mybir as mybir
import concourse.tile as tile
from concourse import bacc
from concourse import bass_utils

D = 1024
S = 4096
H = 16
HD = 64
NCORES = 8
HPC = H // NCORES  # heads per core = 2
JW = HPC * HD  # per-core projected width = 128
F16 = mybir.dt.float16
F32 = mybir.dt.float32
EXP_SHIFT = 8.0  # exp(s/8 - 8): keeps E in fp16 range; cancels in softmax

NSQ = S // 512  # 8 query blocks of 512
NSK = S // 128  # 32 key blocks of 128
NDC = D // 128  # 8 contraction chunks


def _emit(tc: tile.TileContext, repeats: int = 1, bench_mode: int = 0, variant: str = "full"):
    nc = tc.nc
    # bench_mode 2: inputs live in Internal DRAM (garbage values) so the
    # benchmark call carries no host->device payload.
    ik = "Internal" if bench_mode == 2 else "ExternalInput"
    qT = nc.dram_tensor("qT", (D, S), F16, kind=ik).ap()
    kT = nc.dram_tensor("kT", (D, S), F16, kind=ik).ap()
    vT = nc.dram_tensor("vT", (D, S), F16, kind=ik).ap()
    wqT = nc.dram_tensor("wqT", (D, JW), F16, kind=ik).ap()
    wkT = nc.dram_tensor("wkT", (D, JW), F16, kind=ik).ap()
    wvT = nc.dram_tensor("wvT", (D, JW), F16, kind=ik).ap()
    woT = nc.dram_tensor("woT", (JW, D), F16, kind=ik).ap()
    # expc[p, 2*i+h] = exp(c_h[i*128+p]) for s-chunk i, head h (ones if b_q=0)
    expc = nc.dram_tensor("expc", (128, 2 * NSK), F32, kind=ik).ap()
    if bench_mode:
        outp = nc.dram_tensor("outp", (S, D), F32, kind="Internal").ap()
        dummy = nc.dram_tensor("bench_out", (1, 128), F32, kind="ExternalOutput").ap()
    else:
        outp = nc.dram_tensor("outp", (S, D), F32, kind="ExternalOutput").ap()
        dummy = None
    for rep in range(repeats):
        _emit_once(tc, qT, kT, vT, wqT, wkT, wvT, woT, expc, outp, rep, variant)
    if dummy is not None:
        with tc.tile_pool(name="dummyp", bufs=1) as dp:
            dt_ = dp.tile([1, 128], F32)
            nc.gpsimd.memset(dt_[:], 1.0)
            nc.sync.dma_start(dummy[:, :], dt_[:])


def _emit_once(tc, qT, kT, vT, wqT, wkT, wvT, woT, expc, outp, rep, variant="full"):
    """Software-pipelined emission: projections are column-streamed and
    interleaved with the attention loop so ScalarE (the bottleneck: 256 exp
    instructions at ~1.03us) starts within a few us and never starves.

    Per sq-block inner loop (sk = key block):
      sk==0:        normalize(sq-1) (recip + K=1 bcast matmul + DVE mul)
      sk in 2,4,6,8: one out-proj s-chunk of sq-1 (4 matmuls + copy + DMA)
      sk==1:        DMA prefetch of q^T columns for sq+1
      sk in 10..13: Q-proj matmuls for sq+1 into a shared "mix" psum slot
    """
    nc = tc.nc
    Exp = mybir.ActivationFunctionType.Exp

    with (
        tc.tile_pool(name=f"weights{rep}", bufs=1) as wpool,
        tc.tile_pool(name=f"big{rep}", bufs=1) as big,
        tc.tile_pool(name=f"chunks{rep}", bufs=24) as chunks,
        tc.tile_pool(name=f"qchunks{rep}", bufs=9) as qchunks,
    ):
        # ---- weights / constants ----
        wq_sb = wpool.tile([128, NDC * JW], F16)
        wk_sb = wpool.tile([128, NDC * JW], F16)
        wv_sb = wpool.tile([128, NDC * JW], F16)
        for c in range(NDC):
            nc.sync.dma_start(wq_sb[:, c * JW : (c + 1) * JW], wqT[c * 128 : (c + 1) * 128, :])
            nc.sync.dma_start(wk_sb[:, c * JW : (c + 1) * JW], wkT[c * 128 : (c + 1) * 128, :])
            nc.sync.dma_start(wv_sb[:, c * JW : (c + 1) * JW], wvT[c * 128 : (c + 1) * 128, :])
        wo0_sb = wpool.tile([64, D], F16)
        wo1_sb = wpool.tile([64, D], F16)
        nc.sync.dma_start(wo0_sb[:], woT[0:64, :])
        nc.sync.dma_start(wo1_sb[:], woT[64:128, :])
        expc_sb = wpool.tile([128, 2 * NSK], F32)
        nc.sync.dma_start(expc_sb[:], expc[:, :])
        expc16 = wpool.tile([128, 2 * NSK], F16)
        nc.vector.tensor_copy(expc16[:], expc_sb[:])
        ones_sb = wpool.tile([128, 64], F32)
        nc.gpsimd.memset(ones_sb[:], 1.0)
        negshift_sb = wpool.tile([128, 1], F32)
        nc.gpsimd.memset(negshift_sb[:], -EXP_SHIFT)
        # tiny dummy exp: pulls the ~2.7us ACT_TABLE_LOAD off the critical
        # path of the first real exp (runs during the DMA/projection head)
        warm_sb = wpool.tile([128, 1], F16)
        nc.scalar.activation(
            warm_sb[:], negshift_sb[:], mybir.ActivationFunctionType.Exp, scale=0.125
        )

        QcT = big.tile([128, S], F16)
        KcT = big.tile([128, S], F16)
        vaug = big.tile([128, NSK * 130], F16)
        CT0 = big.tile([64, S], F16)
        CT1 = big.tile([64, S], F16)

        # ---- attention + lazy Q-proj + pipelined normalize/out-proj ----
        with (
            tc.tile_pool(name=f"stp{rep}", bufs=2, space="PSUM") as stp,
            tc.tile_pool(name=f"up{rep}", bufs=1, space="PSUM") as up,
            tc.tile_pool(name=f"mixp{rep}", bufs=2, space="PSUM") as mixp,
            tc.tile_pool(name=f"ep{rep}", bufs=8) as ep,
            tc.tile_pool(name=f"smallp{rep}", bufs=4) as smallp,
            tc.tile_pool(name=f"ostagep{rep}", bufs=6) as ostagep,
        ):

            def dma_kvblock(b):
                cs = slice(b * 512, (b + 1) * 512)
                kts, vts = [], []
                for c in range(NDC):
                    kt_t = chunks.tile([128, 512], F16, tag="kchunk", name="kt_t")
                    nc.sync.dma_start(kt_t[:], kT[c * 128 : (c + 1) * 128, cs])
                    kts.append(kt_t)
                for c in range(NDC):
                    vt_t = chunks.tile([128, 512], F16, tag="vchunk", name="vt_t")
                    nc.sync.dma_start(vt_t[:], vT[c * 128 : (c + 1) * 128, cs])
                    vts.append(vt_t)
                return kts, vts

            def kproj_mms(b, kts):
                cs = slice(b * 512, (b + 1) * 512)
                kp = mixp.tile([128, 512], F32, tag="mix", name="kp")
                for c in range(NDC):
                    nc.tensor.matmul(
                        kp[:],
                        wk_sb[:, c * JW : (c + 1) * JW],
                        kts[c][:],
                        start=(c == 0),
                        stop=(c == NDC - 1),
                    )
                nc.vector.tensor_copy(KcT[:, cs], kp[:])

            def vproj_mms(b, vts, half=None):
                rng = range(4) if half is None else range(half * 2, half * 2 + 2)
                for ii in rng:
                    i = b * 4 + ii
                    vpt = mixp.tile([128, 512], F32, tag="mix", name="vpt")
                    vps = vpt[:, 0:JW]
                    for c in range(NDC):
                        nc.tensor.matmul(
                            vps,
                            vts[c][:, ii * 128 : (ii + 1) * 128],
                            wv_sb[:, c * JW : (c + 1) * JW],
                            start=(c == 0),
                            stop=(c == NDC - 1),
                        )
                    base = i * 130
                    nc.vector.tensor_scalar_mul(
                        vaug[:, base : base + 64], vps[:, 0:64], expc_sb[:, 2 * i : 2 * i + 1]
                    )
                    nc.vector.tensor_copy(
                        vaug[:, base + 64 : base + 65], expc16[:, 2 * i : 2 * i + 1]
                    )
                    nc.vector.tensor_scalar_mul(
                        vaug[:, base + 65 : base + 129],
                        vps[:, 64:128],
                        expc_sb[:, 2 * i + 1 : 2 * i + 2],
                    )
                    nc.vector.tensor_copy(
                        vaug[:, base + 129 : base + 130], expc16[:, 2 * i + 1 : 2 * i + 2]
                    )

            def dma_qblock(sq):
                ts = []
                cs = slice(sq * 512, (sq + 1) * 512)
                for c in range(NDC):
                    qt_t = qchunks.tile([128, 512], F16, tag="qchunk", name="qt_t")
                    nc.sync.dma_start(qt_t[:], qT[c * 128 : (c + 1) * 128, cs])
                    ts.append(qt_t)
                return ts

            def qproj_mms(sq, qtiles):
                qp = mixp.tile([128, 512], F32, tag="mix", name="qp")
                for c in range(NDC):
                    nc.tensor.matmul(
                        qp[:],
                        wq_sb[:, c * JW : (c + 1) * JW],
                        qtiles[c][:],
                        start=(c == 0),
                        stop=(c == NDC - 1),
                    )
                nc.vector.tensor_copy(QcT[:, sq * 512 : (sq + 1) * 512], qp[:])

            def normalize(sq, U0, U1):
                sqs = slice(sq * 512, (sq + 1) * 512)
                for h, U, CT in ((0, U0, CT0), (1, U1, CT1)):
                    rr = smallp.tile([65, 512], F32, tag="rr", name="rr")
                    nc.vector.reciprocal(rr[64:65, :], U[64:65, :])
                    bc = mixp.tile([128, 512], F32, tag="mix", name="bc")
                    nc.tensor.matmul(
                        bc[0:64, :], ones_sb[64:65, 0:64], rr[64:65, :], start=True, stop=True
                    )
                    bc_sb = smallp.tile([64, 512], F32, tag="bcsb", name="bc_sb")
                    nc.vector.tensor_copy(bc_sb[:], bc[0:64, :])
                    nc.vector.tensor_mul(CT[:, sqs], U[0:64, :], bc_sb[:])

            def outproj_chunk(sq, scl):
                s0 = sq * 4 + scl
                scs = slice(s0 * 128, (s0 + 1) * 128)
                for nh in range(2):
                    po = mixp.tile([128, 512], F32, tag="mix", name="po")
                    nc.tensor.matmul(
                        po[:],
                        CT0[:, scs],
                        wo0_sb[:, nh * 512 : (nh + 1) * 512],
                        start=True,
                        stop=False,
                    )
                    nc.tensor.matmul(
                        po[:],
                        CT1[:, scs],
                        wo1_sb[:, nh * 512 : (nh + 1) * 512],
                        start=False,
                        stop=True,
                    )
                    ost = ostagep.tile([128, 512], F32, tag="ost", name="ost")
                    nc.vector.tensor_copy(ost[:], po[:])
                    nc.sync.dma_start(outp[scs, nh * 512 : (nh + 1) * 512], ost[:])

            qtiles = dma_qblock(0)
            kts0, vts0 = dma_kvblock(0)
            kproj_mms(0, kts0)
            qproj_mms(0, qtiles)
            vproj_mms(0, vts0)
            kvts = {}
            prev_norm = None  # (sq, U0, U1) awaiting normalize + outproj

            for sq in range(NSQ):
                sqs = slice(sq * 512, (sq + 1) * 512)
                U0 = up.tile([65, 512], F32, tag="u0", name="U0")
                U1 = up.tile([65, 512], F32, tag="u1", name="U1")

                def emit_av(k, e_t, U0=U0, U1=U1):
                    nc.tensor.matmul(
                        U0[:],
                        vaug[:, k * 130 : k * 130 + 65],
                        e_t[:, 0:512],
                        start=(k == 0),
                        stop=(k == NSK - 1),
                    )
                    if variant == "noav":
                        return
                    nc.tensor.matmul(
                        U1[:],
                        vaug[:, k * 130 + 65 : k * 130 + 130],
                        e_t[:, 512:1024],
                        start=(k == 0),
                        stop=(k == NSK - 1),
                    )

                elist = []
                qtiles = None
                AV_LAG = 2
                for sk in range(NSK):
                    sks = slice(sk * 128, (sk + 1) * 128)
                    st = stp.tile([128, 1024], F32, name="st")
                    nc.tensor.matmul(
                        st[:, 0:512],
                        KcT[0:64, sks],
                        QcT[0:64, sqs],
                        start=True,
                        stop=True,
                        tile_position=(0, 0),
                    )
                    nc.tensor.matmul(
                        st[:, 512:1024],
                        KcT[64:128, sks],
                        QcT[64:128, sqs],
                        start=True,
                        stop=True,
                        tile_position=(64, 0),
                    )
                    e_t = ep.tile([128, 1024], F16, tag="e", name="e_t")
                    nc.scalar.activation(e_t[:], st[:], Exp, scale=0.125, bias=negshift_sb[:])

                    if sq == 0 and sk < 28:
                        b = sk // 4 + 1
                        if sk % 4 == 0:
                            kvts[b] = dma_kvblock(b)
                        elif sk % 4 == 1:
                            kproj_mms(b, kvts[b][0])
                        elif sk % 4 == 2:
                            vproj_mms(b, kvts[b][1], half=0)
                        elif sk % 4 == 3:
                            vproj_mms(b, kvts.pop(b)[1], half=1)
                    if sk == 0 and prev_norm is not None:
                        if variant == "noav":
                            pv = prev_norm[1]
                            sink = smallp.tile([65, 512], F32, tag="rr", name="sink")
                            nc.vector.tensor_copy(sink[:], pv[:])
                        else:
                            normalize(*prev_norm)
                    if sk in (8, 11, 14, 17) and prev_norm is not None and variant != "noav":
                        outproj_chunk(prev_norm[0], (sk - 8) // 3)
                    if sk == 1 and sq + 1 < NSQ:
                        qtiles = dma_qblock(sq + 1)
                    if sk == 4 and sq + 1 < NSQ:
                        qproj_mms(sq + 1, qtiles)

                    elist.append((sk, e_t))
                    if sk >= AV_LAG:
                        emit_av(*elist[sk - AV_LAG])
                for k in range(NSK - AV_LAG, NSK):
                    emit_av(*elist[k])
                prev_norm = (sq, U0, U1)

            if variant == "noav":
                sink = smallp.tile([65, 512], F32, tag="rr", name="sink")
                nc.vector.tensor_copy(sink[:], prev_norm[1][:])
                so = smallp.tile([1, 128], F32, tag="so", name="so")
                nc.vector.tensor_copy(so[:], sink[0:1, 0:128])
                nc.sync.dma_start(outp[0:1, 0:128], so[:])
            else:
                normalize(*prev_norm)
                for scl in range(4):
                    outproj_chunk(prev_norm[0], scl)


_CACHE = {}


def _build(repeats: int = 1, bench_mode: int = 0, variant: str = "full"):
    key = (repeats, bench_mode, variant)
    if key in _CACHE:
        return _CACHE[key]
    nc = bacc.Bacc("TRN2", target_bir_lowering=False, debug=False, num_devices=NCORES)
    with tile.TileContext(nc) as tc:
        _emit(tc, repeats=repeats, bench_mode=bench_mode, variant=variant)
    nc.compile()
    _CACHE[key] = nc
    return nc


def _prep_inputs(q, k, v, w_q, b_q, w_k, b_k, w_v, b_v, w_o, b_o):
    """Build the 8 per-core input maps (and the host-side output correction)."""
    q2 = np.asarray(q, np.float32).reshape(S, D)
    k2 = np.asarray(k, np.float32).reshape(S, D)
    v2 = np.asarray(v, np.float32).reshape(S, D)
    qTh = np.ascontiguousarray(q2.T).astype(np.float16)
    kTh = np.ascontiguousarray(k2.T).astype(np.float16)
    vTh = np.ascontiguousarray(v2.T).astype(np.float16)

    in_maps = []
    for c in range(NCORES):
        rows = slice(c * JW, (c + 1) * JW)
        m = {
            "qT": qTh,
            "kT": kTh,
            "vT": vTh,
            "wqT": np.ascontiguousarray(np.asarray(w_q)[rows, :].T).astype(np.float16),
            "wkT": np.ascontiguousarray(np.asarray(w_k)[rows, :].T).astype(np.float16),
            "wvT": np.ascontiguousarray(np.asarray(w_v)[rows, :].T).astype(np.float16),
            "woT": np.ascontiguousarray(np.asarray(w_o)[:, rows].T).astype(np.float16),
        }
        # per-column softmax offset from b_q (exact): c_h[j] = K_h[j] . b_q_h
        ex = np.ones((128, 2 * NSK), np.float32)
        if np.any(np.asarray(b_q) != 0.0):
            for h in range(HPC):
                hrows = slice(c * JW + h * HD, c * JW + (h + 1) * HD)
                u = np.asarray(w_k)[hrows, :].T @ np.asarray(b_q)[hrows]  # [D]
                ch = k2 @ u + float(np.asarray(b_k)[hrows] @ np.asarray(b_q)[hrows])
                # scores are scaled by 1/sqrt(HD) before exp, so the offset is too
                ch = ch / np.sqrt(HD)
                ex[:, h::2] = (
                    np.exp(ch.astype(np.float64)).astype(np.float32).reshape(NSK, 128).T
                )
        m["expc"] = ex
        in_maps.append(m)

    corr = (np.asarray(w_o, np.float64) @ np.asarray(b_v, np.float64)) + np.asarray(
        b_o, np.float64
    )
    return in_maps, corr.astype(np.float32)


def kernel_with_results(trace=False, **inputs):
    nc = _build()
    in_maps, corr = _prep_inputs(**inputs)
    res = bass_utils.run_bass_kernel_spmd(
        nc, in_maps, core_ids=list(range(NCORES)), trace=trace
    )
    out = np.zeros((S, D), np.float32)
    for c in range(NCORES):
        out += res.results[c]["outp"]
    out += corr[None, :]
    return out.reshape(1, S, D), res


def kernel(**inputs):
    out, _ = kernel_with_results(trace=False, **inputs)
    return out



# revision 10
# speedup vs baseline: 1.3243x; 1.3243x over previous
"""Multi-head attention (B=1, S=4096, D=1024, H=16) on 8 TRN2 NeuronCores.

Sharding: tensor-parallel over heads — 2 heads per core. W_q/W_k/W_v are
column-sharded (rows of the torch-style weight), W_o row-sharded; each core
produces a partial output [S, D] (fp16) and the host sums the 8 partials.

Per-core dataflow (fp16 data, fp32 PSUM accumulation):
  1. QcT/KcT = [128(2h*64), 4096]: Qc^T = Wq_c @ q^T via chunked matmuls.
  2. Vc (natural [s, j] layout) + augmented ones column per head (gives the
     softmax denominator for free through the AV matmul).
  3. Attention in S^T layout: St[sk, sq] = Kh^T.T @ Qh^T (two heads packed via
     PE row-tiling), exp via ScalarE directly from PSUM (scale=1/8 and a
     constant -8 shift folded in; max-subtraction skipped — scores bounded).
  4. AV in NATURAL layout: ctx[sq128, 65] += E_chunk^T.T @ [Vh | 1], i.e. the
     E tile is the stationary operand (65-row moving) — 2.5x cheaper on PE
     than the U-layout AV (stationary loads are hidden by streaming).
  5. Normalize with per-partition reciprocal multiply (denominator = col 64),
     PE-transpose the natural ctx into CT = ctx^T [128, S] fp16 (both heads
     stacked), out-proj = single 128-contraction matmul per 512-col half.

Softmax bias subtlety: a nonzero b_q adds a per-COLUMN (sk) offset
c_h[sk] = K_h[sk]·b_q_h to the scores (row-constant terms cancel in softmax).
Handled exactly by scaling V rows and the ones-column by exp(c_h[sk])
(host passes the tiny exp(c) vectors; all-ones when b_q == 0).
b_v/b_o contribute a constant row vector w_o@b_v + b_o added on the host
(softmax rows sum to 1).
"""

import sys

sys.path.insert(0, "/opt/trn_rl_repo")

import numpy as np

import concourse.bass as bass
import concourse.mybir as mybir
import concourse.tile as tile
from concourse import bacc
from concourse import bass_utils
from concourse.masks import make_identity

D = 1024
S = 4096
H = 16
HD = 64
NCORES = 8
HPC = H // NCORES  # heads per core = 2
JW = HPC * HD  # per-core projected width = 128
F16 = mybir.dt.float16
F32 = mybir.dt.float32
EXP_SHIFT = 8.0  # exp(s/8 - 8): keeps E in fp16 range; cancels in softmax

NSQ = S // 512  # 8 query blocks of 512
NSK = S // 128  # 32 key blocks of 128
NDC = D // 128  # 8 contraction chunks


def _emit(tc: tile.TileContext, repeats: int = 1, bench_mode: int = 0, variant: str = "full"):
    nc = tc.nc
    # bench_mode 2: inputs live in Internal DRAM (garbage values) so the
    # benchmark call carries no host->device payload.
    ik = "Internal" if bench_mode == 2 else "ExternalInput"
    qT = nc.dram_tensor("qT", (D, S), F16, kind=ik).ap()
    kT = nc.dram_tensor("kT", (D, S), F16, kind=ik).ap()
    vT = nc.dram_tensor("vT", (D, S), F16, kind=ik).ap()
    wqT = nc.dram_tensor("wqT", (D, JW), F16, kind=ik).ap()
    wkT = nc.dram_tensor("wkT", (D, JW), F16, kind=ik).ap()
    wvT = nc.dram_tensor("wvT", (D, JW), F16, kind=ik).ap()
    woT = nc.dram_tensor("woT", (JW, D), F16, kind=ik).ap()
    # expc[p, 2*i+h] = exp(c_h[i*128+p]) for s-chunk i, head h (ones if b_q=0)
    expc = nc.dram_tensor("expc", (128, 2 * NSK), F32, kind=ik).ap()
    if bench_mode:
        outp = nc.dram_tensor("outp", (S, D), F16, kind="Internal").ap()
        dummy = nc.dram_tensor("bench_out", (1, 128), F32, kind="ExternalOutput").ap()
    else:
        outp = nc.dram_tensor("outp", (S, D), F16, kind="ExternalOutput").ap()
        dummy = None
    for rep in range(repeats):
        _emit_once(tc, qT, kT, vT, wqT, wkT, wvT, woT, expc, outp, rep, variant)
    if dummy is not None:
        with tc.tile_pool(name="dummyp", bufs=1) as dp:
            dt_ = dp.tile([1, 128], F32)
            nc.gpsimd.memset(dt_[:], 1.0)
            nc.sync.dma_start(dummy[:, :], dt_[:])


def _emit_once(tc, qT, kT, vT, wqT, wkT, wvT, woT, expc, outp, rep, variant="full"):
    """Software-pipelined emission. ScalarE (exp, ~1us per (sq,sk) tile) is
    the pacing engine; projections stream through sq-block 0's iterations and
    the normalize/transpose/out-proj of block b runs inside block b+1's loop.
    """
    nc = tc.nc
    Exp = mybir.ActivationFunctionType.Exp

    with (
        tc.tile_pool(name=f"weights{rep}", bufs=1) as wpool,
        tc.tile_pool(name=f"big{rep}", bufs=1) as big,
        tc.tile_pool(name=f"chunks{rep}", bufs=24) as chunks,
        tc.tile_pool(name=f"qchunks{rep}", bufs=9) as qchunks,
    ):
        # ---- weights / constants ----
        wq_sb = wpool.tile([128, NDC * JW], F16)
        wk_sb = wpool.tile([128, NDC * JW], F16)
        wv_sb = wpool.tile([128, NDC * JW], F16)
        for c in range(NDC):
            nc.sync.dma_start(wq_sb[:, c * JW : (c + 1) * JW], wqT[c * 128 : (c + 1) * 128, :])
            nc.sync.dma_start(wk_sb[:, c * JW : (c + 1) * JW], wkT[c * 128 : (c + 1) * 128, :])
            nc.sync.dma_start(wv_sb[:, c * JW : (c + 1) * JW], wvT[c * 128 : (c + 1) * 128, :])
        wo_sb = wpool.tile([128, D], F16)
        nc.sync.dma_start(wo_sb[:], woT[:, :])
        expc_sb = wpool.tile([128, 2 * NSK], F32)
        nc.sync.dma_start(expc_sb[:], expc[:, :])
        expc16 = wpool.tile([128, 2 * NSK], F16)
        nc.vector.tensor_copy(expc16[:], expc_sb[:])
        ident = wpool.tile([128, 128], F16)
        make_identity(nc, ident)
        negshift_sb = wpool.tile([128, 1], F32)
        nc.gpsimd.memset(negshift_sb[:], -EXP_SHIFT)
        # tiny dummy exp: pulls the ~2.7us ACT_TABLE_LOAD off the critical
        # path of the first real exp (runs during the DMA/projection head)
        warm_sb = wpool.tile([128, 1], F16)
        nc.scalar.activation(
            warm_sb[:], negshift_sb[:], mybir.ActivationFunctionType.Exp, scale=0.125
        )

        QcT = big.tile([128, S], F16)
        KcT = big.tile([128, S], F16)
        vaug = big.tile([128, NSK * 130], F16)
        CT = big.tile([128, S], F16)

        # ---- attention + lazy Q-proj + pipelined normalize/out-proj ----
        with (
            tc.tile_pool(name=f"stp{rep}", bufs=2, space="PSUM") as stp,
            tc.tile_pool(name=f"ctxp{rep}", bufs=1, space="PSUM") as ctxp,
            tc.tile_pool(name=f"mixp{rep}", bufs=2, space="PSUM") as mixp,
            tc.tile_pool(name=f"ep{rep}", bufs=8) as ep,
            tc.tile_pool(name=f"smallp{rep}", bufs=2) as smallp,
            tc.tile_pool(name=f"ctxnp{rep}", bufs=2) as ctxnp,
            tc.tile_pool(name=f"ostagep{rep}", bufs=6) as ostagep,
        ):

            def dma_kvblock(b):
                cs = slice(b * 512, (b + 1) * 512)
                kts, vts = [], []
                for c in range(NDC):
                    kt_t = chunks.tile([128, 512], F16, tag="kchunk", name="kt_t")
                    nc.sync.dma_start(kt_t[:], kT[c * 128 : (c + 1) * 128, cs])
                    kts.append(kt_t)
                for c in range(NDC):
                    vt_t = chunks.tile([128, 512], F16, tag="vchunk", name="vt_t")
                    nc.sync.dma_start(vt_t[:], vT[c * 128 : (c + 1) * 128, cs])
                    vts.append(vt_t)
                return kts, vts

            def kproj_mms(b, kts):
                cs = slice(b * 512, (b + 1) * 512)
                kp = mixp.tile([128, 512], F32, tag="mix", name="kp")
                for c in range(NDC):
                    nc.tensor.matmul(
                        kp[:],
                        wk_sb[:, c * JW : (c + 1) * JW],
                        kts[c][:],
                        start=(c == 0),
                        stop=(c == NDC - 1),
                    )
                nc.vector.tensor_copy(KcT[:, cs], kp[:])

            def vproj_mms(b, vts, half=None):
                rng = range(4) if half is None else range(half * 2, half * 2 + 2)
                for ii in rng:
                    i = b * 4 + ii
                    vpt = mixp.tile([128, 512], F32, tag="mix", name="vpt")
                    vps = vpt[:, 0:JW]
                    for c in range(NDC):
                        nc.tensor.matmul(
                            vps,
                            vts[c][:, ii * 128 : (ii + 1) * 128],
                            wv_sb[:, c * JW : (c + 1) * JW],
                            start=(c == 0),
                            stop=(c == NDC - 1),
                        )
                    base = i * 130
                    nc.vector.tensor_scalar_mul(
                        vaug[:, base : base + 64], vps[:, 0:64], expc_sb[:, 2 * i : 2 * i + 1]
                    )
                    nc.vector.tensor_copy(
                        vaug[:, base + 64 : base + 65], expc16[:, 2 * i : 2 * i + 1]
                    )
                    nc.vector.tensor_scalar_mul(
                        vaug[:, base + 65 : base + 129],
                        vps[:, 64:128],
                        expc_sb[:, 2 * i + 1 : 2 * i + 2],
                    )
                    nc.vector.tensor_copy(
                        vaug[:, base + 129 : base + 130], expc16[:, 2 * i + 1 : 2 * i + 2]
                    )

            def dma_qblock(sq):
                ts = []
                cs = slice(sq * 512, (sq + 1) * 512)
                for c in range(NDC):
                    qt_t = qchunks.tile([128, 512], F16, tag="qchunk", name="qt_t")
                    nc.sync.dma_start(qt_t[:], qT[c * 128 : (c + 1) * 128, cs])
                    ts.append(qt_t)
                return ts

            def qproj_mms(sq, qtiles):
                qp = mixp.tile([128, 512], F32, tag="mix", name="qp")
                for c in range(NDC):
                    nc.tensor.matmul(
                        qp[:],
                        wq_sb[:, c * JW : (c + 1) * JW],
                        qtiles[c][:],
                        start=(c == 0),
                        stop=(c == NDC - 1),
                    )
                nc.vector.tensor_copy(QcT[:, sq * 512 : (sq + 1) * 512], qp[:])

            # ---- prev-block chore emitters (normalize/transpose/outproj) ----
            def chore_recip(st8):
                sq, ctxPs, r, ctxn = st8
                for h in range(2):
                    for c in range(4):
                        j = h * 4 + c
                        nc.vector.reciprocal(
                            r[:, j : j + 1], ctxPs[h][:, c * 66 + 64 : c * 66 + 65]
                        )

            def chore_tsmul(st8, j):
                sq, ctxPs, r, ctxn = st8
                h, c = divmod(j, 4)
                # ctxn is chunk-major: [sq128, c*128 + h*64] so one [128,128]
                # transpose per sq-chunk covers both heads (full partitions)
                nc.vector.tensor_scalar_mul(
                    ctxn[:, c * 128 + h * 64 : c * 128 + h * 64 + 64],
                    ctxPs[h][:, c * 66 : c * 66 + 64],
                    r[:, j : j + 1],
                )

            def chore_transpose(st8, c):
                sq, ctxPs, r, ctxn = st8
                tp = mixp.tile([128, 128], F16, tag="mix", name="tp")
                nc.tensor.transpose(tp[:], ctxn[:, c * 128 : (c + 1) * 128], ident[:])
                return tp

            def chore_ctcopy(st8, c, tp):
                sq, ctxPs, r, ctxn = st8
                nc.vector.tensor_copy(
                    CT[:, sq * 512 + c * 128 : sq * 512 + (c + 1) * 128], tp[:]
                )

            def chore_outproj(st8, scl, nh):
                sq, ctxPs, r, ctxn = st8
                s0 = sq * 4 + scl
                scs = slice(s0 * 128, (s0 + 1) * 128)
                po = mixp.tile([128, 512], F32, tag="mix", name="po")
                nc.tensor.matmul(
                    po[:],
                    CT[:, scs],
                    wo_sb[:, nh * 512 : (nh + 1) * 512],
                    start=True,
                    stop=True,
                )
                ost = ostagep.tile([128, 512], F16, tag="ost", name="ost")
                nc.vector.tensor_copy(ost[:], po[:])
                nc.sync.dma_start(outp[scs, nh * 512 : (nh + 1) * 512], ost[:])

            qtiles = dma_qblock(0)
            kts0, vts0 = dma_kvblock(0)
            kproj_mms(0, kts0)
            qproj_mms(0, qtiles)
            vproj_mms(0, vts0)
            kvts = {}
            prev = None  # chore state of the previous sq block
            AV_LAG = 3

            for sq in range(NSQ):
                sqs = slice(sq * 512, (sq + 1) * 512)
                ctxPa = ctxp.tile([128, 264], F32, tag="ca", name="ctxPa")
                ctxPb = ctxp.tile([128, 264], F32, tag="cb", name="ctxPb")
                ctxPs = (ctxPa, ctxPb)

                def emit_av(k, e_t, ctxPs=ctxPs):
                    # One start=True per PSUM bank (first chunk, k=0): on HW
                    # first_mm clears has_written for the whole bank, so a
                    # per-chunk start would erase sibling chunks' k=0 sums.
                    # Chunks 1-3 first-write with start=False (overwrite-or-
                    # accumulate-onto-cleared — correct either way).
                    for h in range(2):
                        for c in range(4):
                            nc.tensor.matmul(
                                ctxPs[h][:, c * 66 : c * 66 + 65],
                                e_t[:, h * 512 + c * 128 : h * 512 + (c + 1) * 128],
                                vaug[:, k * 130 + h * 65 : k * 130 + (h + 1) * 65],
                                start=(k == 0 and c == 0),
                                stop=(k == NSK - 1 and c == 3),
                                skip_group_check=True,
                            )

                elist = []
                qtiles = None
                tps = {}
                for sk in range(NSK):
                    sks = slice(sk * 128, (sk + 1) * 128)
                    st = stp.tile([128, 1024], F32, name="st")
                    nc.tensor.matmul(
                        st[:, 0:512],
                        KcT[0:64, sks],
                        QcT[0:64, sqs],
                        start=True,
                        stop=True,
                        tile_position=(0, 0),
                    )
                    nc.tensor.matmul(
                        st[:, 512:1024],
                        KcT[64:128, sks],
                        QcT[64:128, sqs],
                        start=True,
                        stop=True,
                        tile_position=(64, 0),
                    )
                    e_t = ep.tile([128, 1024], F16, tag="e", name="e_t")
                    nc.scalar.activation(e_t[:], st[:], Exp, scale=0.125, bias=negshift_sb[:])

                    # ---- interleaved chores ----
                    if sq == 0 and sk < 28:
                        b = sk // 4 + 1
                        if sk % 4 == 0:
                            kvts[b] = dma_kvblock(b)
                        elif sk % 4 == 1:
                            kproj_mms(b, kvts[b][0])
                        elif sk % 4 == 2:
                            vproj_mms(b, kvts[b][1], half=0)
                        elif sk % 4 == 3:
                            vproj_mms(b, kvts.pop(b)[1], half=1)
                    if prev is not None:
                        if sk == 0:
                            chore_recip(prev)
                            for j in range(4):
                                chore_tsmul(prev, j)
                        elif sk == 1:
                            for j in range(4, 8):
                                chore_tsmul(prev, j)
                        elif sk == 2:
                            tps[0] = chore_transpose(prev, 0)
                        elif sk in (3, 4, 5):
                            # keep alloc->read distance < mix pool bufs (2):
                            # each tp is consumed before 2 more mix allocations
                            c0 = sk - 3
                            chore_ctcopy(prev, c0, tps.pop(c0))
                            tps[c0 + 1] = chore_transpose(prev, c0 + 1)
                        elif sk == 6:
                            chore_ctcopy(prev, 3, tps.pop(3))
                        elif sk >= 8 and sk < 24 and sk % 2 == 0:
                            idx = (sk - 8) // 2
                            chore_outproj(prev, idx // 2, idx % 2)
                    if sk == 1 and sq + 1 < NSQ:
                        qtiles = dma_qblock(sq + 1)
                    if sk == 7 and sq + 1 < NSQ:
                        qproj_mms(sq + 1, qtiles)

                    elist.append((sk, e_t))
                    if sk >= AV_LAG:
                        emit_av(*elist[sk - AV_LAG])
                for k in range(NSK - AV_LAG, NSK):
                    emit_av(*elist[k])

                r = smallp.tile([128, 8], F32, tag="r", name="r")
                ctxn = ctxnp.tile([128, 512], F16, tag="ctxn", name="ctxn")
                prev = (sq, ctxPs, r, ctxn)

            # ---- epilogue: chores for the last block ----
            chore_recip(prev)
            for j in range(8):
                chore_tsmul(prev, j)
            tps = {}
            for c in range(4):
                tps[c] = chore_transpose(prev, c)
                chore_ctcopy(prev, c, tps.pop(c))
            for scl in range(4):
                for nh in range(2):
                    chore_outproj(prev, scl, nh)

            if variant == "debug":
                nc_ = tc.nc
                ctxPd = nc_.dram_tensor("ctxPd", (128, 528), F32, kind="ExternalOutput").ap()
                with tc.tile_pool(name="dbgp", bufs=1) as dbgp:
                    dbg_sb = dbgp.tile([128, 528], F32)
                    nc_.vector.tensor_copy(dbg_sb[:, 0:264], prev[1][0][:])
                    nc_.vector.tensor_copy(dbg_sb[:, 264:528], prev[1][1][:])
                    nc_.sync.dma_start(ctxPd[:, :], dbg_sb[:])
                CTd = nc_.dram_tensor("CTd", (128, S), F16, kind="ExternalOutput").ap()
                QcTd = nc_.dram_tensor("QcTd", (128, S), F16, kind="ExternalOutput").ap()
                KcTd = nc_.dram_tensor("KcTd", (128, S), F16, kind="ExternalOutput").ap()
                vaugd = nc_.dram_tensor(
                    "vaugd", (128, NSK * 130), F16, kind="ExternalOutput"
                ).ap()
                ctxnd = nc_.dram_tensor("ctxnd", (128, 512), F16, kind="ExternalOutput").ap()
                nc_.sync.dma_start(CTd[:, :], CT[:])
                nc_.sync.dma_start(QcTd[:, :], QcT[:])
                nc_.sync.dma_start(KcTd[:, :], KcT[:])
                nc_.sync.dma_start(vaugd[:, :], vaug[:])
                nc_.sync.dma_start(ctxnd[:, :], prev[3][:])


_CACHE = {}


def _build(repeats: int = 1, bench_mode: int = 0, variant: str = "full"):
    key = (repeats, bench_mode, variant)
    if key in _CACHE:
        return _CACHE[key]
    nc = bacc.Bacc("TRN2", target_bir_lowering=False, debug=False, num_devices=NCORES)
    with tile.TileContext(nc) as tc:
        _emit(tc, repeats=repeats, bench_mode=bench_mode, variant=variant)
    nc.compile()
    _CACHE[key] = nc
    return nc


def _prep_inputs(q, k, v, w_q, b_q, w_k, b_k, w_v, b_v, w_o, b_o):
    """Build the 8 per-core input maps (and the host-side output correction)."""
    q2 = np.asarray(q, np.float32).reshape(S, D)
    k2 = np.asarray(k, np.float32).reshape(S, D)
    v2 = np.asarray(v, np.float32).reshape(S, D)
    qTh = np.ascontiguousarray(q2.T).astype(np.float16)
    kTh = np.ascontiguousarray(k2.T).astype(np.float16)
    vTh = np.ascontiguousarray(v2.T).astype(np.float16)

    in_maps = []
    for c in range(NCORES):
        rows = slice(c * JW, (c + 1) * JW)
        m = {
            "qT": qTh,
            "kT": kTh,
            "vT": vTh,
            "wqT": np.ascontiguousarray(np.asarray(w_q)[rows, :].T).astype(np.float16),
            "wkT": np.ascontiguousarray(np.asarray(w_k)[rows, :].T).astype(np.float16),
            "wvT": np.ascontiguousarray(np.asarray(w_v)[rows, :].T).astype(np.float16),
            "woT": np.ascontiguousarray(np.asarray(w_o)[:, rows].T).astype(np.float16),
        }
        # per-column softmax offset from b_q (exact): c_h[j] = K_h[j] . b_q_h
        ex = np.ones((128, 2 * NSK), np.float32)
        if np.any(np.asarray(b_q) != 0.0):
            for h in range(HPC):
                hrows = slice(c * JW + h * HD, c * JW + (h + 1) * HD)
                u = np.asarray(w_k)[hrows, :].T @ np.asarray(b_q)[hrows]  # [D]
                ch = k2 @ u + float(np.asarray(b_k)[hrows] @ np.asarray(b_q)[hrows])
                # scores are scaled by 1/sqrt(HD) before exp, so the offset is too
                ch = ch / np.sqrt(HD)
                ex[:, h::2] = (
                    np.exp(ch.astype(np.float64)).astype(np.float32).reshape(NSK, 128).T
                )
        m["expc"] = ex
        in_maps.append(m)

    corr = (np.asarray(w_o, np.float64) @ np.asarray(b_v, np.float64)) + np.asarray(
        b_o, np.float64
    )
    return in_maps, corr.astype(np.float32)


def kernel_with_results(trace=False, **inputs):
    nc = _build()
    in_maps, corr = _prep_inputs(**inputs)
    res = bass_utils.run_bass_kernel_spmd(
        nc, in_maps, core_ids=list(range(NCORES)), trace=trace
    )
    out = np.zeros((S, D), np.float32)
    for c in range(NCORES):
        out += res.results[c]["outp"].astype(np.float32)
    out += corr[None, :]
    return out.reshape(1, S, D), res


def kernel(**inputs):
    out, _ = kernel_with_results(trace=False, **inputs)
    return out
